# revision 1
# baseline (speedup 1.0000x reference)
"""MAMGCN submodule kernel for Trainium2, 8-core data-parallel over batch.

Problem (per reference):
  B=16, N=1024, F=64, T=12, K=3, F_OUT=64
  S = softmax_axis1(Vs @ sigmoid(lhs @ rhs^T + bs))
  out = relu(sum_k (cheb_k * S)^T @ x @ Theta_k)

Sharding: batch B=16 split across 8 cores (2 batches/core). All weights
replicated. Each core runs an identical Bass program on its shard.

Layout strategy per core/batch (n = destination node index, m = source):
  - product/P/S/E/A tiles keep m (or i) on partitions, n on free dim.
  - The cheb contraction uses x'-as-stationary matmuls producing
    z'[(t,f), n] transposed; Theta applied via block-diag (128,128)
    stationary; final (t,o)->(o,t)+transpose via PE transpose, with the
    softmax denominator folded in as a per-partition scale on the final
    relu copy.
  - All matmuls run in float32r (full PE rate at free>=256, ~1e-4 rounding).
"""
import numpy as np

import concourse.bass as bass
import concourse.mybir as mybir
import concourse.tile as tile
from concourse import bacc
from concourse.bass_utils import run_bass_kernel_spmd
from concourse.masks import make_identity

F32 = mybir.dt.float32
F32R = mybir.dt.float32r
AL = mybir.AluOpType
AF = mybir.ActivationFunctionType
AX = mybir.AxisListType

B_PER_CORE = 2
N = 1024
F = 64
T = 12
K = 3
FO = 64
NT = N // 128          # 8 n-tiles (128 rows each)
NQ = 4                 # n processed in quarters
HW = N // NQ // 1      # 256 free-dim per quarter
NC_TILES_PER_Q = HW // 128  # 2 c-subtiles of 128 per quarter
TQ = (T * F) // 128    # 6 (t,f)-chunks of x' (each = 2 t-values x 64 f)


def _emit_batch(nc, tc, pools, cst, b, x_d, bs_d, cheb_d, out_d):
    """Emit one batch's pipeline."""
    (stream, bigp, pe_pool, res_pool, psA, psZ, dram_pool) = pools

    # ---- Stage A: load x, reorder to x', attention row features ----
    xprime = bigp.tile([128, NT, T, F], F32R, tag="xp")
    xw1T = stream.tile([F, N], F32R, tag="xw1T", bufs=1)
    rhsBT = stream.tile([T, N], F32R, tag="rhsBT", bufs=1)
    for mi in range(NT):
        xnat = stream.tile([128, F, T], F32, tag="xnat", bufs=2)
        nc.sync.dma_start(out=xnat[:], in_=x_d.ap()[b, mi * 128:(mi + 1) * 128])
        # x' reorder (f,t) -> (t,f), rounded to fp32r
        nc.vector.tensor_copy(xprime[:, mi], xnat[:].rearrange("p f t -> p t f"))
        # xw1[n,f] = sum_t x*W1
        tmp = stream.tile([128, F, T], F32, tag="tmp", bufs=2)
        nc.vector.tensor_mul(tmp[:], xnat[:], cst["w1rep"][:])
        xw1_t = stream.tile([128, F], F32, tag="xw1t")
        nc.vector.tensor_reduce(out=xw1_t[:], in_=tmp[:], op=AL.add, axis=AX.X)
        # rhsB[n,t] = sum_f W3*x
        tmp2 = stream.tile([128, T, F], F32, tag="tmp2", bufs=2)
        nc.vector.tensor_mul(tmp2[:], xprime[:, mi].bitcast(F32), cst["w3rep"][:])
        rhsb_t = stream.tile([128, T], F32, tag="rhsbt")
        nc.vector.tensor_reduce(out=rhsb_t[:], in_=tmp2[:], op=AL.add, axis=AX.X)
        # transpose both to contraction-on-partitions layout
        pst64 = psA.tile([F, 128], F32, tag="a")
        nc.tensor.transpose(pst64[:], xw1_t[:], cst["ident"][:])
        nc.vector.tensor_copy(xw1T[:, mi * 128:(mi + 1) * 128], pst64[:])
        pst12 = psA.tile([T, 128], F32, tag="a")
        nc.tensor.transpose(pst12[:], rhsb_t[:], cst["ident"][:])
        nc.vector.tensor_copy(rhsBT[:, mi * 128:(mi + 1) * 128], pst12[:])

    # ---- Stage B: lhs^T = W2^T @ xw1^T  (12, N) ----
    lhsT_sb = stream.tile([T, N], F32R, tag="lhsT", bufs=1)
    for h in range(2):
        ps_l = psA.tile([T, 512], F32, tag="a")
        nc.tensor.matmul(ps_l[:], cst["w2r"][:], xw1T[:, h * 512:(h + 1) * 512],
                         start=True, stop=True)
        nc.vector.tensor_copy(lhsT_sb[:, h * 512:(h + 1) * 512], ps_l[:])

    # ---- per n-quarter pipeline ----
    for nh in range(NQ):
        HS = slice(nh * HW, (nh + 1) * HW)
        # Stage C: product + bs -> sigmoid -> P
        P_q = pe_pool.tile([128, NT, HW], F32R, tag="P")
        for ii in range(NT):
            ps_p = psA.tile([128, HW], F32, tag="a")
            nc.tensor.matmul(ps_p[:], lhsT_sb[:, ii * 128:(ii + 1) * 128],
                             rhsBT[:, HS], start=True, stop=True)
            bs_t = stream.tile([128, HW], F32, tag="bst")
            nc.sync.dma_start(out=bs_t[:], in_=bs_d.ap()[ii * 128:(ii + 1) * 128, HS])
            sgin = stream.tile([128, HW], F32, tag="sgin")
            nc.vector.tensor_add(sgin[:], ps_p[:], bs_t[:])
            nc.scalar.activation(P_q[:, ii], sgin[:], AF.Sigmoid)
        # Stage D: S = VsT^T @ P ; E = exp(S); colsum via ones-matmul
        E_q = pe_pool.tile([128, NT, HW], F32R, tag="E")
        ps_cs = psA.tile([1, HW], F32, tag="a")
        for ii in range(NT):
            ps_s = psA.tile([128, HW], F32, tag="a")
            for pi in range(NT):
                nc.tensor.matmul(ps_s[:], cst["vsT"][:, pi, ii * 128:(ii + 1) * 128],
                                 P_q[:, pi], start=(pi == 0), stop=(pi == NT - 1))
            nc.scalar.activation(E_q[:, ii], ps_s[:], AF.Exp)
            nc.tensor.matmul(ps_cs[:], cst["ones_r"][:], E_q[:, ii],
                             start=(ii == 0), stop=(ii == NT - 1))
        # softmax denominator reciprocal, then scatter to partitions
        cs_row = stream.tile([1, HW], F32, tag="cs")
        nc.vector.tensor_copy(cs_row[:], ps_cs[:])
        rc_d = dram_pool.tile([HW], F32, tag="rcd", name="rc_d")
        nc.sync.dma_start(out=rc_d.rearrange("(a b) -> a b", a=1),
                          in_=cs_row[:])
        rc_sc = stream.tile([128, NC_TILES_PER_Q], F32, tag="rcsc")
        nc.sync.dma_start(out=rc_sc[:],
                          in_=rc_d.rearrange("(c p) -> p c", p=128))
        recip_sb = stream.tile([128, NC_TILES_PER_Q], F32, tag="recip")
        nc.vector.reciprocal(recip_sb[:], rc_sc[:])
        # Stage E: A_k = cheb_k * E
        A_q = pe_pool.tile([128, K, NT, HW], F32R, tag="A", bufs=2)
        for mi in range(NT):
            for k in range(K):
                cheb_t = stream.tile([128, HW], F32, tag="chebt")
                nc.sync.dma_start(
                    out=cheb_t[:],
                    in_=cheb_d.ap()[k, mi * 128:(mi + 1) * 128, HS])
                nc.vector.tensor_mul(A_q[:, k, mi], cheb_t[:],
                                     E_q[:, mi].bitcast(F32))
        # Stage F: z' = x'^T-chunks @ A ; Theta via block-diag; transpose out
        res_tiles = []
        for _c in range(NC_TILES_PER_Q):
            res_c = res_pool.tile([128, FO, T], F32, tag="res", name=f"res{_c}")
            res_tiles.append(res_c)
        for q in range(TQ):
            ps_z = psZ.tile([128, K, HW], F32, tag="z", padded_shape=[128, K, 512])
            for mi in range(NT):
                for k in range(K):
                    nc.tensor.matmul(ps_z[:, k], xprime[:, mi, 2 * q:2 * q + 2, :],
                                     A_q[:, k, mi],
                                     start=(mi == 0), stop=(mi == NT - 1))
            ps_o = psA.tile([128, HW], F32, tag="a")
            for k in range(K):
                zs = stream.tile([128, HW], F32R, tag="zs")
                nc.scalar.copy(zs[:], ps_z[:, k])
                nc.tensor.matmul(ps_o[:], cst["thbd"][:, k, :], zs[:],
                                 start=(k == 0), stop=(k == K - 1))
            os_t = stream.tile([128, HW], F32, tag="os")
            nc.scalar.copy(os_t[:], ps_o[:])
            for c in range(NC_TILES_PER_Q):
                ps_tr = psA.tile([128, 128], F32, tag="a")
                nc.tensor.transpose(ps_tr[:], os_t[:, c * 128:(c + 1) * 128],
                                    cst["ident"][:])
                nc.vector.tensor_scalar(
                    out=res_tiles[c][:, :, 2 * q:2 * q + 2],
                    in0=ps_tr[:].rearrange("p (dt o) -> p o dt", o=FO),
                    scalar1=recip_sb[:, c:c + 1],
                    scalar2=0.0,
                    op0=AL.mult,
                    op1=AL.max,
                )
        for c in range(NC_TILES_PER_Q):
            nt_i = nh * NC_TILES_PER_Q + c
            nc.sync.dma_start(
                out=out_d.ap()[b, nt_i * 128:(nt_i + 1) * 128],
                in_=res_tiles[c][:])


def build_nc(repeat=1):
    nc = bacc.Bacc("TRN2", target_bir_lowering=False, debug=False, num_devices=8)
    x_d = nc.dram_tensor("x", [B_PER_CORE, N, F, T], F32, kind="ExternalInput")
    w1_d = nc.dram_tensor("W1", [T], F32, kind="ExternalInput")
    w2_d = nc.dram_tensor("W2", [F, T], F32, kind="ExternalInput")
    w3_d = nc.dram_tensor("W3", [F], F32, kind="ExternalInput")
    bs_d = nc.dram_tensor("bs", [N, N], F32, kind="ExternalInput")
    vs_d = nc.dram_tensor("Vs", [N, N], F32, kind="ExternalInput")
    cheb_d = nc.dram_tensor("cheb", [K, N, N], F32, kind="ExternalInput")
    th_d = nc.dram_tensor("Theta", [K, F, FO], F32, kind="ExternalInput")
    out_d = nc.dram_tensor("out", [B_PER_CORE, N, FO, T], F32,
                           kind="ExternalOutput")

    with tile.TileContext(nc) as tc:
        with (
            tc.tile_pool(name="consts", bufs=1) as consts,
            tc.tile_pool(name="stream", bufs=3) as stream,
            tc.tile_pool(name="bigp", bufs=1) as bigp,
            tc.tile_pool(name="pe", bufs=2) as pe_pool,
            tc.tile_pool(name="res", bufs=4) as res_pool,
            tc.tile_pool(name="dram", bufs=2, space="DRAM") as dram_pool,
            tc.tile_pool(name="psA", bufs=4, space="PSUM") as psA,
            tc.tile_pool(name="psZ", bufs=1, space="PSUM") as psZ,
        ):
            cst = {}
            ident = consts.tile([128, 128], F32)
            make_identity(nc, ident[:])
            cst["ident"] = ident
            # ones vectors (fp32r via rounding copy)
            onesf = consts.tile([128, 1], F32)
            nc.vector.memset(onesf[:], 1.0)
            ones_r = consts.tile([128, 1], F32R)
            nc.vector.tensor_copy(ones_r[:], onesf[:])
            cst["ones_r"] = ones_r
            # broadcast W1 / W3 replicas
            w1rep = consts.tile([128, F, T], F32)
            nc.gpsimd.dma_start(
                out=w1rep[:],
                in_=bass.AP(tensor=w1_d, offset=0, ap=[[0, 128], [0, F], [1, T]]))
            cst["w1rep"] = w1rep
            w3rep = consts.tile([128, T, F], F32)
            nc.gpsimd.dma_start(
                out=w3rep[:],
                in_=bass.AP(tensor=w3_d, offset=0, ap=[[0, 128], [0, T], [1, F]]))
            cst["w3rep"] = w3rep
            # W2 (f, t) fp32r
            w2f = consts.tile([F, T], F32)
            nc.sync.dma_start(out=w2f[:], in_=w2_d.ap())
            w2r = consts.tile([F, T], F32R)
            nc.vector.tensor_copy(w2r[:], w2f[:])
            cst["w2r"] = w2r
            # block-diagonal Theta (128, K, 128)
            thbd_f = consts.tile([128, K, 128], F32)
            nc.vector.memset(thbd_f[:], 0.0)
            for k in range(K):
                nc.sync.dma_start(out=thbd_f[0:F, k, 0:FO], in_=th_d.ap()[k])
                nc.sync.dma_start(out=thbd_f[F:128, k, FO:128], in_=th_d.ap()[k])
            thbd = consts.tile([128, K, 128], F32R)
            nc.vector.tensor_copy(thbd[:], thbd_f[:])
            cst["thbd"] = thbd
            # VsT (p-partitioned Vs transpose), fp32r
            vsT = consts.tile([128, NT, N], F32R)
            for pi in range(NT):
                for ii in range(NT):
                    vtmp = stream.tile([128, 128], F32, tag="vtmp", bufs=2)
                    nc.sync.dma_start(
                        out=vtmp[:],
                        in_=vs_d.ap()[ii * 128:(ii + 1) * 128,
                                      pi * 128:(pi + 1) * 128])
                    ps_v = psA.tile([128, 128], F32, tag="a")
                    nc.tensor.transpose(ps_v[:], vtmp[:], ident[:])
                    nc.vector.tensor_copy(vsT[:, pi, ii * 128:(ii + 1) * 128],
                                          ps_v[:])
            cst["vsT"] = vsT

            pools = (stream, bigp, pe_pool, res_pool, psA, psZ, dram_pool)
            for _ in range(repeat):
                for b in range(B_PER_CORE):
                    _emit_batch(nc, tc, pools, cst, b, x_d, bs_d, cheb_d, out_d)
    nc.compile()
    return nc


_RUNNER_CACHE = {}


def _make_runner(repeat=1):
    """Build the Bass program once and wrap it in a persistent jitted
    shard_map executable so repeat calls skip recompile/reload."""
    import jax
    from jax.sharding import Mesh, PartitionSpec
    from jax.experimental.shard_map import shard_map
    from concourse import bass2jax, mybir as _mybir

    nc = build_nc(repeat)
    bass2jax.install_neuronx_cc_hook()

    part_name = nc.partition_id_tensor.name if nc.partition_id_tensor else None
    in_names = []
    out_names = []
    out_avals = []
    zero_outs = []
    for alloc in nc.m.functions[0].allocations:
        if not isinstance(_mybir.MemoryLocationSet, type) or not isinstance(
                alloc, _mybir.MemoryLocationSet):
            continue
        name = alloc.memorylocations[0].name
        if alloc.kind == "ExternalInput":
            if name != part_name:
                in_names.append(name)
        elif alloc.kind == "ExternalOutput":
            out_names.append(name)
            shape = tuple(alloc.tensor_shape)
            dtype = _mybir.dt.np(alloc.dtype)
            out_avals.append(jax.core.ShapedArray(shape, dtype))
            zero_outs.append(np.zeros(shape, dtype))
    n_params = len(in_names)
    all_names = in_names + out_names
    if part_name is not None:
        all_names = all_names + [part_name]

    def _body(*args):
        operands = list(args)
        if part_name is not None:
            operands.append(bass2jax.partition_id_tensor())
        outs = bass2jax._bass_exec_p.bind(
            *operands,
            out_avals=tuple(out_avals),
            in_names=tuple(all_names),
            out_names=tuple(out_names),
            lowering_input_output_aliases=(),
            sim_require_finite=False,
            sim_require_nnan=False,
            nc=nc,
        )
        return tuple(outs)

    n_cores = 8
    devices = jax.devices()[:n_cores]
    mesh = Mesh(np.asarray(devices), ("core",))
    in_specs = tuple(
        PartitionSpec("core") if name == "x" else PartitionSpec()
        for name in in_names
    ) + (PartitionSpec("core"),) * len(out_names)
    out_specs = (PartitionSpec("core"),) * len(out_names)
    sharded = jax.jit(
        shard_map(_body, mesh=mesh, in_specs=in_specs, out_specs=out_specs,
                  check_rep=False),
        keep_unused=True,
    )
    return nc, sharded, in_names, out_names, zero_outs, n_cores, mesh


def _get_runner(repeat=1):
    if repeat not in _RUNNER_CACHE:
        _RUNNER_CACHE[repeat] = _make_runner(repeat)
    return _RUNNER_CACHE[repeat]


def kernel(x, W1, W2, W3, bs, Vs, cheb, Theta, repeat=1):
    x = np.asarray(x, dtype=np.float32)
    full = {
        "W1": np.asarray(W1, dtype=np.float32),
        "W2": np.asarray(W2, dtype=np.float32),
        "W3": np.asarray(W3, dtype=np.float32),
        "bs": np.asarray(bs, dtype=np.float32).reshape(N, N),
        "Vs": np.asarray(Vs, dtype=np.float32),
        "cheb": np.asarray(cheb, dtype=np.float32),
        "Theta": np.asarray(Theta, dtype=np.float32),
    }
    nc, sharded, in_names, out_names, zero_outs, n_cores, mesh = _get_runner(repeat)
    ops = _staged_ops(x, full, in_names, zero_outs, n_cores)
    out_arrs = sharded(*ops)
    out = np.asarray(out_arrs[out_names.index("out")])
    return out.reshape(16, N, FO, T)


def _staged_ops(x, full, in_names, zero_outs, n_cores):
    ops = []
    for name in in_names:
        if name == "x":
            ops.append(np.ascontiguousarray(x.reshape(n_cores * B_PER_CORE, N, F, T)))
        else:
            ops.append(full[name])
    for z in zero_outs:
        ops.append(np.zeros((n_cores * z.shape[0], *z.shape[1:]), z.dtype))
    return ops


def _bench_setup(inputs, repeat):
    import jax
    from jax.sharding import NamedSharding, PartitionSpec
    x = np.asarray(inputs["x"], dtype=np.float32)
    full = {k: np.asarray(v, dtype=np.float32) for k, v in inputs.items() if k != "x"}
    full["bs"] = full["bs"].reshape(N, N)
    nc, sharded, in_names, out_names, zero_outs, n_cores, mesh = _get_runner(repeat)
    ops = _staged_ops(x, full, in_names, zero_outs, n_cores)
    sh_core = NamedSharding(mesh, PartitionSpec("core"))
    sh_rep = NamedSharding(mesh, PartitionSpec())
    shardings = [sh_core if name == "x" else sh_rep for name in in_names]
    shardings += [sh_core] * len(zero_outs)
    dev_ops = [jax.device_put(o, s_) for o, s_ in zip(ops, shardings)]
    jax.block_until_ready(sharded(*dev_ops))
    return sharded, dev_ops


def bench_pair(inputs, rep_a=1, rep_b=9, iters=20):
    """Interleaved device-resident timing of two repeat variants.
    Returns (best_a, best_b) seconds — interleaving cancels slow drift in the
    fixed dispatch overhead."""
    import time as _time
    import jax
    sh_a, ops_a = _bench_setup(inputs, rep_a)
    sh_b, ops_b = _bench_setup(inputs, rep_b)
    best_a = best_b = float("inf")
    for _ in range(iters):
        t0 = _time.time()
        jax.block_until_ready(sh_a(*ops_a))
        best_a = min(best_a, _time.time() - t0)
        t0 = _time.time()
        jax.block_until_ready(sh_b(*ops_b))
        best_b = min(best_b, _time.time() - t0)
    return best_a, best_b



# revision 6
# speedup vs baseline: 10.5109x; 10.5109x over previous
"""MAMGCN submodule kernel for Trainium2, 8-core data-parallel over batch.

Problem (per reference):
  B=16, N=1024, F=64, T=12, K=3, F_OUT=64
  S = softmax_axis1(Vs @ sigmoid(lhs @ rhs^T + bs))
  out = relu(sum_k (cheb_k * S)^T @ x @ Theta_k)

Sharding: batch B=16 split across 8 cores (2 batches/core). All weights
replicated. Each core runs an identical Bass program on its shard.

v2 design notes:
  - All large matmuls use 512-wide moving operands (one PSUM bank), with
    the attention path (Vs/P/E/cheb/A/x') in bf16 (errors ~0.5%, well
    under the 2e-2 gate) so the hot set fits in SBUF and FWL kicks in.
  - Per batch: stage A (x load/reorder + small reductions), stage P
    (product + bs -> sigmoid, sigmoids grouped to avoid act-table
    thrash), then per j-half: S-accumulate -> exp -> colsum, A_k =
    cheb_k * E (split across DVE and Pool engines), z via x'-stationary
    matmuls, Theta via block-diag stationary accumulated across k in
    PSUM, then PE transpose + fused relu*recip writeback.
  - Softmax denominator folded into the final relu as a per-partition
    scale (partition = destination node j after the transpose).
"""
import numpy as np

import concourse.bass as bass
import concourse.mybir as mybir
import concourse.tile as tile
from concourse import bacc
from concourse.masks import make_identity

F32 = mybir.dt.float32
F32R = mybir.dt.float32r
BF16 = mybir.dt.bfloat16
AL = mybir.AluOpType
AF = mybir.ActivationFunctionType
AX = mybir.AxisListType

B_PER_CORE = 2
N = 1024
F = 64
T = 12
K = 3
FO = 64
NT = N // 128           # 8 node tiles
JH = 512                # j processed in halves of 512
NJH = N // JH           # 2
TF = (T * F) // 128     # 6 (t,f)-chunks (each = 2 t-values x 64 f)


def _emit_batch(nc, pools, cst, b, x_d, cheb_d, out_d):
    (sbp, psMain, psOut, dram_pool) = pools
    ident = cst["ident"]
    identr = cst["identr"]

    # ---------------- Stage A: x load + row features ----------------
    # (The x' reorder for the z matmuls is emitted after stage P so that the
    # next batch's attention pipeline never waits on the z-phase of the
    # previous batch for an SBUF slot.)
    xw1T = sbp.tile([F, N], F32R, tag="xw1T", bufs=1, name="xw1T")
    rhsBT = sbp.tile([T, N], F32R, tag="rhsBT", bufs=1, name="rhsBT")
    xnats = []
    for i in range(NT):
        xnat = sbp.tile([128, F, T], F32, tag="xnat", bufs=NT, name="xnat")
        nc.sync.dma_start(out=xnat[:], in_=x_d.ap()[b, i * 128:(i + 1) * 128])
        xnats.append(xnat)
        # xw1[n,f] = sum_t x*W1   (Pool engine)
        tmpA = sbp.tile([128, F, T], BF16, tag="tmpA", bufs=1, name="tmpA")
        nc.gpsimd.tensor_mul(tmpA[:], xnat[:], cst["w1rep"][:])
        xw1_i = sbp.tile([128, F], F32, tag="xw1i", bufs=2, name="xw1_i")
        nc.vector.tensor_reduce(out=xw1_i[:], in_=tmpA[:], op=AL.add, axis=AX.X)
        # rhsB[n,t] = sum_f W3*x   (DVE, strided read from native x)
        tmpB = sbp.tile([128, T, F], BF16, tag="tmpB", bufs=1, name="tmpB")
        nc.vector.tensor_mul(tmpB[:], xnat[:].rearrange("p f t -> p t f"),
                             cst["w3rep"][:])
        rhsB_i = sbp.tile([128, T], F32, tag="rhsBi", bufs=2, name="rhsB_i")
        nc.vector.tensor_reduce(out=rhsB_i[:], in_=tmpB[:], op=AL.add, axis=AX.X)
        # transpose both to contraction-on-partitions layout
        ps_t1 = psMain.tile([F, 128], F32, tag="m", name="ps_t1")
        nc.tensor.transpose(ps_t1[:], xw1_i[:], ident[:])
        nc.scalar.copy(xw1T[:, i * 128:(i + 1) * 128], ps_t1[:])
        ps_t2 = psMain.tile([T, 128], F32, tag="m", name="ps_t2")
        nc.tensor.transpose(ps_t2[:], rhsB_i[:], ident[:])
        nc.scalar.copy(rhsBT[:, i * 128:(i + 1) * 128], ps_t2[:])

    # lhsT[t, u] = sum_f W2[f,t] * xw1T[f, u]
    lhsT_sb = sbp.tile([T, N], F32R, tag="lhsT", bufs=1, name="lhsT_sb")
    for h in range(2):
        ps_l = psMain.tile([T, JH], F32, tag="m", name="ps_l")
        nc.tensor.matmul(ps_l[:], cst["w2r"][:], xw1T[:, h * JH:(h + 1) * JH],
                         start=True, stop=True)
        nc.scalar.copy(lhsT_sb[:, h * JH:(h + 1) * JH], ps_l[:])

    # ---------------- Stage P: product + bs -> sigmoid (grouped) ------------
    P_sb = sbp.tile([128, NT, N], BF16, tag="P", bufs=1, name="P_sb")
    for jh in range(NJH):
        JS = slice(jh * JH, (jh + 1) * JH)
        for u in range(NT):
            ps_p = psMain.tile([128, JH], F32, tag="m", name="ps_p")
            nc.tensor.matmul(ps_p[:], lhsT_sb[:, u * 128:(u + 1) * 128],
                             rhsBT[:, JS], start=True, stop=True)
            sg = sbp.tile([128, JH], F32, tag="sg", bufs=2, name="sg")
            nc.vector.tensor_add(sg[:], ps_p[:], cst["bs"][:, u, JS])
            nc.scalar.activation(P_sb[:, u, JS], sg[:], AF.Sigmoid)

    # x' reorder (f,t) -> (t,f), bf16 — stationary operand of the z matmuls
    xprime = sbp.tile([128, NT, T, F], BF16, tag="xp", bufs=1, name="xprime")
    for i in range(NT):
        nc.vector.tensor_copy(xprime[:, i],
                              xnats[i][:].rearrange("p f t -> p t f"))

    # ---------------- per j-half: S, exp, colsum, A, z, Theta, out ----------
    for jh in range(NJH):
        JS = slice(jh * JH, (jh + 1) * JH)
        # cheb prefetch for all 3 k of this j-half (SWDGE cast f32->bf16)
        cheb_t = []
        for k in range(K):
            ct = sbp.tile([128, NT, JH], BF16, tag="cheb", bufs=2, name="cheb_t")
            nc.gpsimd.dma_start(
                out=ct[:],
                in_=cheb_d.ap()[k, :, JS].rearrange("(i p) n -> p i n", p=128))
            cheb_t.append(ct)

        # S = Vs^T-stationary accumulation; E = exp(S); A_k = cheb_k * E
        E_q = sbp.tile([128, NT, JH], BF16, tag="E", bufs=2, name="E_q")
        A_q = [sbp.tile([128, NT, JH], BF16, tag="A", bufs=3, name=f"A_q{k}")
               for k in range(K)]
        for i in range(NT):
            ps_s = psMain.tile([128, JH], F32, tag="m", name="ps_s")
            for u in range(NT):
                nc.tensor.matmul(ps_s[:], cst["vsT"][:, u, i * 128:(i + 1) * 128],
                                 P_sb[:, u, JS],
                                 start=(u == 0), stop=(u == NT - 1))
            nc.scalar.activation(E_q[:, i], ps_s[:], AF.Exp)
            # A_k tiles for this i (split across DVE / Pool)
            nc.vector.tensor_mul(A_q[0][:, i], cheb_t[0][:, i], E_q[:, i])
            nc.gpsimd.tensor_mul(A_q[1][:, i], cheb_t[1][:, i], E_q[:, i])
            nc.gpsimd.tensor_mul(A_q[2][:, i], cheb_t[2][:, i], E_q[:, i])

        # colsum d[j] = sum_i E[i, j] via ones-stationary matmul
        ps_cs = psMain.tile([128, JH], F32, tag="m", name="ps_cs")
        for i in range(NT):
            nc.tensor.matmul(ps_cs[0:1, :], cst["ones_bf"][:], E_q[:, i],
                             start=(i == 0), stop=(i == NT - 1))
        cs_sb = sbp.tile([1, JH], F32, tag="cs", bufs=2, name="cs_sb")
        nc.scalar.copy(cs_sb[:], ps_cs[0:1, :])
        rc_sb = sbp.tile([1, JH], F32, tag="rc", bufs=2, name="rc_sb")
        nc.vector.reciprocal(rc_sb[:], cs_sb[:])
        rc_d = dram_pool.tile([JH], F32, tag="rcd", name="rc_d")
        nc.sync.dma_start(out=rc_d.rearrange("(a b) -> a b", a=1), in_=rc_sb[:])
        recip_sb = sbp.tile([128, JH // 128], F32, tag="recip", bufs=2,
                            name="recip_sb")
        nc.sync.dma_start(out=recip_sb[:], in_=rc_d.rearrange("(c p) -> p c", p=128))

        # z' = x'-stationary matmuls; Theta via block-diag accumulated over k
        psOut_t = psOut.tile([128, TF, JH], F32, tag="out", name="psOut_t")
        theta_pending = []  # software-pipelined Theta matmuls
        for k in range(K):
            for tf in range(TF):
                ps_z = psMain.tile([128, JH], F32, tag="m", name="ps_z")
                for i in range(NT):
                    nc.tensor.matmul(ps_z[:],
                                     xprime[:, i].rearrange("p t f -> p (t f)")
                                     [:, tf * 128:(tf + 1) * 128],
                                     A_q[k][:, i],
                                     start=(i == 0), stop=(i == NT - 1))
                if theta_pending:
                    theta_pending.pop(0)()
                z_sb = sbp.tile([128, JH], F32R, tag="zsb", bufs=3, name="z_sb")
                nc.scalar.copy(z_sb[:], ps_z[:])

                def _mk(k=k, tf=tf, z_sb=z_sb, psOut_t=psOut_t):
                    def _do():
                        nc.tensor.matmul(psOut_t[:, tf], cst["thbd"][:, k, :],
                                         z_sb[:], start=(k == 0), stop=(k == K - 1))
                    return _do
                theta_pending.append(_mk())
        for fn in theta_pending:
            fn()

        # drain psOut -> SBUF, transpose, fused relu * recip writeback
        oT = sbp.tile([128, TF, JH], F32R, tag="oT", bufs=1, name="oT")
        for tf in range(TF):
            nc.scalar.copy(oT[:, tf], psOut_t[:, tf])
        for js in range(JH // 128):
            res = sbp.tile([128, FO, T], F32, tag="res", bufs=2, name="res")
            for g in range(2):
                ps_tr = psMain.tile([128, 384], F32R, tag="m", name="ps_tr")
                for q in range(3):
                    nc.tensor.transpose(
                        ps_tr[:, q * 128:(q + 1) * 128],
                        oT[:, g * 3 + q, js * 128:(js + 1) * 128], identr[:])
                nc.scalar.activation(
                    res[:].rearrange("p o (gg q dt) -> p gg q dt o", gg=2, q=3,
                                     dt=2)[:, g],
                    ps_tr[:].rearrange("p (q dt o) -> p q dt o", q=3, o=FO),
                    AF.Relu, scale=recip_sb[:, js:js + 1])
            nj = jh * (JH // 128) + js
            nc.sync.dma_start(out=out_d.ap()[b, nj * 128:(nj + 1) * 128],
                              in_=res[:])


def build_nc(repeat=1):
    nc = bacc.Bacc("TRN2", target_bir_lowering=False, debug=False, num_devices=8)
    x_d = nc.dram_tensor("x", [B_PER_CORE, N, F, T], F32, kind="ExternalInput")
    w1_d = nc.dram_tensor("W1", [T], F32, kind="ExternalInput")
    w2_d = nc.dram_tensor("W2", [F, T], F32, kind="ExternalInput")
    w3_d = nc.dram_tensor("W3", [F], F32, kind="ExternalInput")
    bs_d = nc.dram_tensor("bs", [N, N], F32, kind="ExternalInput")
    vs_d = nc.dram_tensor("Vs", [N, N], F32, kind="ExternalInput")
    cheb_d = nc.dram_tensor("cheb", [K, N, N], F32, kind="ExternalInput")
    th_d = nc.dram_tensor("Theta", [K, F, FO], F32, kind="ExternalInput")
    out_d = nc.dram_tensor("out", [B_PER_CORE, N, FO, T], F32,
                           kind="ExternalOutput")

    with tile.TileContext(nc) as tc:
        with (
            tc.tile_pool(name="consts", bufs=1) as consts,
            tc.tile_pool(name="sbp", bufs=1) as sbp,
            tc.tile_pool(name="dram", bufs=2, space="DRAM") as dram_pool,
            tc.tile_pool(name="psMain", bufs=2, space="PSUM") as psMain,
            tc.tile_pool(name="psOut", bufs=1, space="PSUM") as psOut,
        ):
            cst = {}
            ident = consts.tile([128, 128], F32)
            make_identity(nc, ident[:])
            cst["ident"] = ident
            identr = consts.tile([128, 128], F32R)
            nc.vector.tensor_copy(identr[:], ident[:])
            cst["identr"] = identr
            onesf = consts.tile([128, 1], F32)
            nc.vector.memset(onesf[:], 1.0)
            ones_bf = consts.tile([128, 1], BF16)
            nc.vector.tensor_copy(ones_bf[:], onesf[:])
            cst["ones_bf"] = ones_bf
            # broadcast W1 / W3 replicas
            w1rep = consts.tile([128, F, T], F32)
            nc.gpsimd.dma_start(
                out=w1rep[:],
                in_=bass.AP(tensor=w1_d, offset=0, ap=[[0, 128], [0, F], [1, T]]))
            cst["w1rep"] = w1rep
            w3rep = consts.tile([128, T, F], F32)
            nc.gpsimd.dma_start(
                out=w3rep[:],
                in_=bass.AP(tensor=w3_d, offset=0, ap=[[0, 128], [0, T], [1, F]]))
            cst["w3rep"] = w3rep
            # W2 (f, t) fp32r
            w2f = consts.tile([F, T], F32)
            nc.sync.dma_start(out=w2f[:], in_=w2_d.ap())
            w2r = consts.tile([F, T], F32R)
            nc.vector.tensor_copy(w2r[:], w2f[:])
            cst["w2r"] = w2r
            # bs resident, bf16 (cast during SWDGE DMA)
            bs_sb = consts.tile([128, NT, N], BF16, name="bs_sb")
            nc.gpsimd.dma_start(
                out=bs_sb[:],
                in_=bs_d.ap().rearrange("(u p) n -> p u n", p=128))
            cst["bs"] = bs_sb
            # block-diagonal Theta (128, K, 128) fp32r
            thbd_f = consts.tile([128, K, 128], F32)
            nc.vector.memset(thbd_f[:], 0.0)
            for k in range(K):
                nc.sync.dma_start(out=thbd_f[0:F, k, 0:FO], in_=th_d.ap()[k])
                nc.sync.dma_start(out=thbd_f[F:128, k, FO:128], in_=th_d.ap()[k])
            thbd = consts.tile([128, K, 128], F32R)
            nc.vector.tensor_copy(thbd[:], thbd_f[:])
            cst["thbd"] = thbd
            # VsT (u-partitioned Vs transpose), bf16
            vsT = consts.tile([128, NT, N], BF16, name="vsT")
            for ut in range(NT):
                for it in range(NT):
                    vtmp = sbp.tile([128, 128], F32, tag="vtmp", bufs=2,
                                    name="vtmp")
                    nc.sync.dma_start(
                        out=vtmp[:],
                        in_=vs_d.ap()[it * 128:(it + 1) * 128,
                                      ut * 128:(ut + 1) * 128])
                    ps_v = psMain.tile([128, 128], F32, tag="m", name="ps_v")
                    nc.tensor.transpose(ps_v[:], vtmp[:], ident[:])
                    nc.scalar.copy(vsT[:, ut, it * 128:(it + 1) * 128], ps_v[:])
            cst["vsT"] = vsT

            pools = (sbp, psMain, psOut, dram_pool)
            for _ in range(repeat):
                for b in range(B_PER_CORE):
                    _emit_batch(nc, pools, cst, b, x_d, cheb_d, out_d)
    nc.compile()
    return nc


_RUNNER_CACHE = {}


def _make_runner(repeat=1):
    """Build the Bass program once and wrap it in a persistent jitted
    shard_map executable so repeat calls skip recompile/reload."""
    import jax
    from jax.sharding import Mesh, PartitionSpec
    from jax.experimental.shard_map import shard_map
    from concourse import bass2jax, mybir as _mybir

    nc = build_nc(repeat)
    bass2jax.install_neuronx_cc_hook()

    part_name = nc.partition_id_tensor.name if nc.partition_id_tensor else None
    in_names = []
    out_names = []
    out_avals = []
    zero_outs = []
    for alloc in nc.m.functions[0].allocations:
        if not isinstance(_mybir.MemoryLocationSet, type) or not isinstance(
                alloc, _mybir.MemoryLocationSet):
            continue
        name = alloc.memorylocations[0].name
        if alloc.kind == "ExternalInput":
            if name != part_name:
                in_names.append(name)
        elif alloc.kind == "ExternalOutput":
            out_names.append(name)
            shape = tuple(alloc.tensor_shape)
            dtype = _mybir.dt.np(alloc.dtype)
            out_avals.append(jax.core.ShapedArray(shape, dtype))
            zero_outs.append(np.zeros(shape, dtype))
    n_params = len(in_names)
    all_names = in_names + out_names
    if part_name is not None:
        all_names = all_names + [part_name]

    def _body(*args):
        operands = list(args)
        if part_name is not None:
            operands.append(bass2jax.partition_id_tensor())
        outs = bass2jax._bass_exec_p.bind(
            *operands,
            out_avals=tuple(out_avals),
            in_names=tuple(all_names),
            out_names=tuple(out_names),
            lowering_input_output_aliases=(),
            sim_require_finite=False,
            sim_require_nnan=False,
            nc=nc,
        )
        return tuple(outs)

    n_cores = 8
    devices = jax.devices()[:n_cores]
    mesh = Mesh(np.asarray(devices), ("core",))
    in_specs = tuple(
        PartitionSpec("core") if name == "x" else PartitionSpec()
        for name in in_names
    ) + (PartitionSpec("core"),) * len(out_names)
    out_specs = (PartitionSpec("core"),) * len(out_names)
    sharded = jax.jit(
        shard_map(_body, mesh=mesh, in_specs=in_specs, out_specs=out_specs,
                  check_rep=False),
        keep_unused=True,
    )
    return nc, sharded, in_names, out_names, zero_outs, n_cores, mesh


def _get_runner(repeat=1):
    if repeat not in _RUNNER_CACHE:
        _RUNNER_CACHE[repeat] = _make_runner(repeat)
    return _RUNNER_CACHE[repeat]


def kernel(x, W1, W2, W3, bs, Vs, cheb, Theta, repeat=1):
    x = np.asarray(x, dtype=np.float32)
    full = {
        "W1": np.asarray(W1, dtype=np.float32),
        "W2": np.asarray(W2, dtype=np.float32),
        "W3": np.asarray(W3, dtype=np.float32),
        "bs": np.asarray(bs, dtype=np.float32).reshape(N, N),
        "Vs": np.asarray(Vs, dtype=np.float32),
        "cheb": np.asarray(cheb, dtype=np.float32),
        "Theta": np.asarray(Theta, dtype=np.float32),
    }
    nc, sharded, in_names, out_names, zero_outs, n_cores, mesh = _get_runner(repeat)
    ops = _staged_ops(x, full, in_names, zero_outs, n_cores)
    out_arrs = sharded(*ops)
    out = np.asarray(out_arrs[out_names.index("out")])
    return out.reshape(16, N, FO, T)


def _staged_ops(x, full, in_names, zero_outs, n_cores):
    ops = []
    for name in in_names:
        if name == "x":
            ops.append(np.ascontiguousarray(x.reshape(n_cores * B_PER_CORE, N, F, T)))
        else:
            ops.append(full[name])
    for z in zero_outs:
        ops.append(np.zeros((n_cores * z.shape[0], *z.shape[1:]), z.dtype))
    return ops


def _bench_setup(inputs, repeat):
    import jax
    from jax.sharding import NamedSharding, PartitionSpec
    x = np.asarray(inputs["x"], dtype=np.float32)
    full = {k: np.asarray(v, dtype=np.float32) for k, v in inputs.items() if k != "x"}
    full["bs"] = full["bs"].reshape(N, N)
    nc, sharded, in_names, out_names, zero_outs, n_cores, mesh = _get_runner(repeat)
    ops = _staged_ops(x, full, in_names, zero_outs, n_cores)
    sh_core = NamedSharding(mesh, PartitionSpec("core"))
    sh_rep = NamedSharding(mesh, PartitionSpec())
    shardings = [sh_core if name == "x" else sh_rep for name in in_names]
    shardings += [sh_core] * len(zero_outs)
    dev_ops = [jax.device_put(o, s_) for o, s_ in zip(ops, shardings)]
    jax.block_until_ready(sharded(*dev_ops))
    return sharded, dev_ops


def bench_pair(inputs, rep_a=1, rep_b=9, iters=20):
    """Interleaved device-resident timing of two repeat variants.
    Returns (best_a, best_b) seconds — interleaving cancels slow drift in the
    fixed dispatch overhead."""
    import time as _time
    import jax
    sh_a, ops_a = _bench_setup(inputs, rep_a)
    sh_b, ops_b = _bench_setup(inputs, rep_b)
    best_a = best_b = float("inf")
    for _ in range(iters):
        t0 = _time.time()
        jax.block_until_ready(sh_a(*ops_a))
        best_a = min(best_a, _time.time() - t0)
        t0 = _time.time()
        jax.block_until_ready(sh_b(*ops_b))
        best_b = min(best_b, _time.time() - t0)
    return best_a, best_b
